# revision 3
# baseline (speedup 1.0000x reference)
"""Multi-head attention (B=4, N=2048, DIM=768, H=8, DH=96) on 8 TRN2 NeuronCores.

Sharding: data-parallel over (batch, query-half) — core c handles batch c//2,
query rows [(c%2)*1024, (c%2+1)*1024). Each core computes K/V for its full
batch (duplicated across the 2 cores sharing a batch): no collectives.

vs previous version: Q/K projections are UNPADDED (6 dense f-tiles of 128
instead of 8 head-padded tiles: -36864 PE cycles/core), then repacked into
per-head [96, N] tiles via SBUF->SBUF DMA on the gpsimd queue. Exp runs on
[128, 1024] PSUM tiles (2 banks) halving ScalarE instruction count; attnV is
software-pipelined one group behind its exp so the PE never waits on ScalarE.
Output is stored bf16 (halves the output DMA tail); host upcasts.

Per-core (all matmuls bf16, fp32 PSUM):
  - K^T/Q^T dense projection: psum [128,512] chunks, evicted bf16 into
    rotating dense tiles ktd[f] [128,2048] / qtd[f] [128,1024] (bufs=3).
  - repack: per (head, col-chunk) DMA pieces ktd[f][r0:r0+a, cols] ->
    kt[h][d0:d0+a, cols]; same for Q. SCALE folded into wq host-side.
  - V in natural space with a 1.0 column appended per head (V|1) so attn@V
    also produces softmax row-sums; fc=0 (head slots 0-3) during head 0,
    fc=1 during heads 1-3.
  - per head, 16 groups (one per key tile t): 2 dots matmuls (qc0/qc1) into
    d_ps [128,1024], one exp [128,1024] -> pt bf16, 2 attnV matmuls
    (emitted one group later) accumulating o_ps[qc] [97,512] over t.
  - normalize: evacuate O' to SBUF, s row to partition 0 (sync DMA), gpsimd
    partition_broadcast, DVE reciprocal_approx_fast, multiply -> on bf16.
  - proj y^T: heads 0-6 accumulated during head 7's attention (bias folded
    into wp row 96 of head 0), head 7 added at the tail; y stored bf16.
"""

import numpy as np
import ml_dtypes

B, N, DIM = 4, 2048, 768
H, DH = 8, 96
NQ = N // 2
SCALE = DH ** -0.5
NCORES = 8
CT = DIM // 128    # 6 contraction chunks
FT = DIM // 128    # 6 dense f-tiles
NT = N // 128      # 16 key tiles
NQC = NQ // 512    # 2 query chunks

_CACHE = {}


def _head_spans(h):
    """Dense-row pieces covering rows [96h, 96h+96): (f, r0, a, d0)."""
    lo, hi = DH * h, DH * h + DH
    out = []
    for f in range(lo // 128, (hi - 1) // 128 + 1):
        r0 = max(0, lo - 128 * f)
        r1 = min(128, hi - 128 * f)
        out.append((f, r0, r1 - r0, 128 * f + r0 - lo))
    return out


def _f_hi(h):
    return (DH * h + DH - 1) // 128


def _build():
    import concourse.mybir as mybir
    import concourse.tile as tile
    from concourse import bacc

    f32 = mybir.dt.float32
    bf16 = mybir.dt.bfloat16
    Exp = mybir.ActivationFunctionType.Exp
    mult = mybir.AluOpType.mult
    add = mybir.AluOpType.add

    nc = bacc.Bacc("TRN2", debug=False, num_devices=NCORES)

    xq_d = [nc.dram_tensor(f"xq{i}", [128, CT, 512], bf16, kind="ExternalInput")
            for i in range(4)]
    wk_d = nc.dram_tensor("wk", [128, FT, CT, 128], bf16, kind="ExternalInput")
    wq_d = nc.dram_tensor("wq", [128, FT, CT, 128], bf16, kind="ExternalInput")
    wv_d = nc.dram_tensor("wv", [128, CT, DIM], bf16, kind="ExternalInput")
    wp_d = nc.dram_tensor("wp", [DH + 1, H, DIM], bf16, kind="ExternalInput")
    out_d = nc.dram_tensor("out", [DIM, NQ], bf16, kind="ExternalOutput")

    with tile.TileContext(nc) as tc:
        with (
            tc.tile_pool(name="const", bufs=1) as cpool,
            tc.tile_pool(name="ktdp", bufs=3) as ktd_pool,
            tc.tile_pool(name="qtdp", bufs=3) as qtd_pool,
            tc.tile_pool(name="ktp", bufs=4) as kt_pool,
            tc.tile_pool(name="qtp", bufs=4) as qt_pool,
            tc.tile_pool(name="ptp", bufs=4) as pt_pool,
            tc.tile_pool(name="onp", bufs=16) as on_pool,
            tc.tile_pool(name="smallp", bufs=2) as small_pool,
            tc.tile_pool(name="ysb", bufs=4) as y_pool,
            tc.tile_pool(name="ps_qkv", bufs=2, space="PSUM") as psum_qkv,
            tc.tile_pool(name="ps_d", bufs=2, space="PSUM") as psum_d,
            tc.tile_pool(name="ps_o", bufs=2, space="PSUM") as psum_o,
        ):
            x_sb = [cpool.tile([128, CT, 512], bf16, name=f"x{i}") for i in range(4)]
            wk_sb = cpool.tile([128, FT, CT, 128], bf16, name="wk_sb")
            wq_sb = cpool.tile([128, FT, CT, 128], bf16, name="wq_sb")
            wv_sb = cpool.tile([128, CT, DIM], bf16, name="wv_sb")
            wp_sb = cpool.tile([DH + 1, H, DIM], bf16, name="wp_sb")
            v_sb = [cpool.tile([128, H, DH + 1], bf16, name=f"v{t}") for t in range(NT)]
            y1_sb = [
                [cpool.tile([128, 512], bf16, name=f"y1_{ct}_{qc}") for qc in range(NQC)]
                for ct in range(CT)
            ]

            # input DMAs on the sync queue, ordered by first use
            nc.sync.dma_start(wk_sb[:], wk_d.ap())
            nc.sync.dma_start(x_sb[0][:], xq_d[0].ap())
            nc.sync.dma_start(wq_sb[:], wq_d.ap())
            nc.sync.dma_start(x_sb[1][:], xq_d[1].ap())
            nc.sync.dma_start(wv_sb[:], wv_d.ap())
            nc.sync.dma_start(x_sb[2][:], xq_d[2].ap())
            nc.sync.dma_start(x_sb[3][:], xq_d[3].ap())
            nc.sync.dma_start(wp_sb[:], wp_d.ap())

            for t in range(NT):
                nc.vector.memset(v_sb[t][:, :, DH:DH + 1], 1.0)

            # PE warmup through the input-DMA window (HAM clock at 8/8)
            warm_sb = cpool.tile([128, 128], bf16, name="warm_sb")
            nc.vector.memset(warm_sb[:], 0.0)
            for _ in range(80):
                wps = psum_qkv.tile([128, 512], f32, name="wps", tag="qkvps")
                nc.tensor.matmul(
                    wps[:, 0:128], lhsT=warm_sb[:], rhs=warm_sb[:],
                    start=True, stop=True,
                )

            ktd = {}   # f -> dense K^T tile [128, N]
            qtd = {}   # f -> dense Q^T tile [128, NQ]
            kt = {}    # h -> [96, N]
            qt = {}    # h -> [96, NQ]

            def k_dense(f, nc_):
                if nc_ == 0:
                    ktd[f] = ktd_pool.tile([128, N], bf16, name="ktd", tag="ktd")
                ps = psum_qkv.tile([128, 512], f32, name="kps", tag="qkvps")
                for ct in range(CT):
                    nc.tensor.matmul(
                        ps,
                        lhsT=wk_sb[:, f, ct, :],
                        rhs=x_sb[nc_][:, ct, :],
                        start=(ct == 0),
                        stop=(ct == CT - 1),
                    )
                nc.vector.tensor_copy(
                    out=ktd[f][:, nc_ * 512:(nc_ + 1) * 512], in_=ps[:]
                )

            def q_dense(f, qc):
                if qc == 0:
                    qtd[f] = qtd_pool.tile([128, NQ], bf16, name="qtd", tag="qtd")
                ps = psum_qkv.tile([128, 512], f32, name="qps", tag="qkvps")
                for ct in range(CT):
                    nc.tensor.matmul(
                        ps,
                        lhsT=wq_sb[:, f, ct, :],
                        rhs=x_sb[qc][:, ct, :],
                        start=(ct == 0),
                        stop=(ct == CT - 1),
                    )
                nc.vector.tensor_copy(
                    out=qtd[f][:, qc * 512:(qc + 1) * 512], in_=ps[:]
                )

            def kp(h, nc_):
                """Repack kt[h] cols [512*nc_, 512*nc_+512) from dense tiles."""
                if h not in kt:
                    kt[h] = kt_pool.tile([DH, N], bf16, name="kt", tag="kt")
                c0, c1 = nc_ * 512, (nc_ + 1) * 512
                for (f, r0, a, d0) in _head_spans(h):
                    nc.gpsimd.dma_start(
                        kt[h][d0:d0 + a, c0:c1], ktd[f][r0:r0 + a, c0:c1]
                    )

            def qp(h, qc):
                if h not in qt:
                    qt[h] = qt_pool.tile([DH, NQ], bf16, name="qt", tag="qt")
                c0, c1 = qc * 512, (qc + 1) * 512
                for (f, r0, a, d0) in _head_spans(h):
                    nc.gpsimd.dma_start(
                        qt[h][d0:d0 + a, c0:c1], qtd[f][r0:r0 + a, c0:c1]
                    )

            def v_chunk(t, fc):
                ps = psum_qkv.tile([128, 512], f32, name="vps", tag="qkvps")
                vps = ps[:, 0:4 * DH]
                for ct in range(CT):
                    nc.tensor.matmul(
                        vps,
                        lhsT=x_sb[t // 4][:, ct, (t % 4) * 128:(t % 4 + 1) * 128],
                        rhs=wv_sb[:, ct, fc * 4 * DH:(fc + 1) * 4 * DH],
                        start=(ct == 0),
                        stop=(ct == CT - 1),
                    )
                nc.vector.tensor_copy(
                    out=v_sb[t][:, fc * 4:(fc + 1) * 4, 0:DH],
                    in_=ps[:, 0:4 * DH],
                )

            on_sb = {}

            def proj06(ct, qc):
                yp = psum_qkv.tile([128, 512], f32, name="yps", tag="qkvps")
                for h in range(7):
                    nc.tensor.matmul(
                        yp,
                        lhsT=wp_sb[:, h, ct * 128:(ct + 1) * 128],
                        rhs=on_sb[(h, qc)][:],
                        start=(h == 0),
                        stop=(h == 6),
                    )
                nc.vector.tensor_copy(out=y1_sb[ct][qc][:], in_=yp[:])

            def proj7(ct, qc):
                yp = psum_qkv.tile([128, 512], f32, name="yp7", tag="qkvps")
                nc.tensor.matmul(
                    yp,
                    lhsT=wp_sb[:, 7, ct * 128:(ct + 1) * 128],
                    rhs=on_sb[(7, qc)][:],
                    start=True,
                    stop=True,
                )
                y_sb = y_pool.tile([128, 512], bf16, name="y", tag="y")
                nc.vector.tensor_tensor(y_sb[:], yp[:], y1_sb[ct][qc][:], add)
                nc.sync.dma_start(
                    out_d.ap()[ct * 128:(ct + 1) * 128, qc * 512:(qc + 1) * 512],
                    y_sb[:],
                )

            def attn_head(h, fillers):
                o_ps = [
                    psum_o.tile([DH + 1, 512], f32, name=f"ops{qc}", tag="ops")
                    for qc in range(NQC)
                ]
                pending = []
                for t in range(NT):
                    d_ps = psum_d.tile([128, 1024], f32, name="dps", tag="dps")
                    nc.tensor.matmul(
                        d_ps[:, 0:512],
                        lhsT=kt[h][:, t * 128:(t + 1) * 128],
                        rhs=qt[h][:, 0:512],
                        start=True, stop=True,
                    )
                    nc.tensor.matmul(
                        d_ps[:, 512:1024],
                        lhsT=kt[h][:, t * 128:(t + 1) * 128],
                        rhs=qt[h][:, 512:1024],
                        start=True, stop=True,
                    )
                    pt = pt_pool.tile([128, 1024], bf16, name="pt", tag="pt")
                    nc.scalar.activation(pt[:], d_ps[:], Exp)
                    # fillers for this slot run before the (delayed) attnV
                    for fn in fillers.get(t, ()):
                        fn()
                    for fn in pending:
                        fn()
                    pending = [
                        (lambda tt=t, pp=pt, qc=qc: nc.tensor.matmul(
                            o_ps[qc],
                            lhsT=v_sb[tt][:, h, :],
                            rhs=pp[:, qc * 512:(qc + 1) * 512],
                            start=(tt == 0),
                            stop=(tt == NT - 1),
                        ))
                        for qc in range(NQC)
                    ]
                for fn in pending:
                    fn()

                o_sts = []
                for qc in range(NQC):
                    o_st = small_pool.tile(
                        [DH + 1, 512], f32, name="ostage", tag="ostage", bufs=7
                    )
                    nc.vector.tensor_copy(out=o_st[:], in_=o_ps[qc][:])
                    o_sts.append(o_st)

                def finish_normalize(h=h, o_sts=o_sts):
                    sbs = []
                    for qc in range(NQC):
                        s0 = small_pool.tile([1, 512], f32, name="s0", tag="s0", bufs=4)
                        nc.sync.dma_start(s0[:], o_sts[qc][DH:DH + 1, :])
                        sb = small_pool.tile([DH, 512], f32, name="sbc", tag="sbc", bufs=4)
                        nc.gpsimd.partition_broadcast(sb[:], s0[:])
                        sbs.append(sb)
                    for qc in range(NQC):
                        sb = sbs[qc]
                        nc.vector.reciprocal_approx_fast(out=sb[:], in_=sb[:])
                        on = on_pool.tile([DH + 1, 512], bf16, name="on", tag="on")
                        on_sb[(h, qc)] = on
                        nc.vector.memset(on[DH:DH + 1, :], 1.0)
                        nc.vector.tensor_tensor(
                            on[0:DH, :], o_sts[qc][0:DH, :], sb[:], mult
                        )

                return finish_normalize

            # ---- preamble: head-0 critical path ----
            k_dense(0, 0)
            kp(0, 0)
            q_dense(0, 0)
            q_dense(0, 1)
            qp(0, 0)
            qp(0, 1)
            v_chunk(0, 0)
            v_chunk(1, 0)

            def mk_fillers(h):
                f = {}

                def addf(slot, fn):
                    f.setdefault(slot, []).append(fn)

                if h == 0:
                    # K f0 cols 1-3 + repack JIT (group 4nc needs piece nc)
                    for n in (1, 2, 3):
                        addf(4 * n - 4, lambda n=n: k_dense(0, n))
                        addf(4 * n - 4, lambda n=n: kp(0, n))
                    # remaining v fc0 chunks
                    for t in range(2, NT):
                        addf(max(0, t - 2), lambda tt=t: v_chunk(tt, 0))
                    # K/Q f1 for head 1 (+ repacks for heads with f_hi==1)
                    for i, n in enumerate((0, 1, 2, 3)):
                        addf(2 * i + 5, lambda n=n: k_dense(1, n))
                        addf(2 * i + 5, lambda n=n: kp(1, n))
                    addf(12, lambda: q_dense(1, 0))
                    addf(12, lambda: qp(1, 0))
                    addf(14, lambda: q_dense(1, 1))
                    addf(14, lambda: qp(1, 1))
                elif h in (1, 2, 3, 4):
                    # dense f = h+1 plus repacks for heads with f_hi == h+1
                    fd = h + 1
                    heads = [hh for hh in range(H) if _f_hi(hh) == fd]
                    for i, n in enumerate((0, 1, 2, 3)):
                        addf(2 * i + 1, lambda n=n: k_dense(fd, n))
                        for hh in heads:
                            addf(2 * i + 1, lambda n=n, hh=hh: kp(hh, n))
                    addf(9, lambda: q_dense(fd, 0))
                    addf(11, lambda: q_dense(fd, 1))
                    for hh in heads:
                        addf(9, lambda hh=hh: qp(hh, 0))
                        addf(11, lambda hh=hh: qp(hh, 1))
                    # v fc1 chunks: 4 per head during heads 1-4 => t 0..15
                    for i in range(4):
                        addf(2 * i + 2, lambda tt=4 * (h - 1) + i: v_chunk(tt, 1))
                if h == 7:
                    for i in range(12):
                        ct, qc = i // 2, i % 2
                        addf(i + 4, lambda c=ct, q=qc: proj06(c, q))
                return f

            from collections import deque
            pending_norm = deque()
            for h in range(H):
                f = mk_fillers(h)
                if h == H - 1:
                    while pending_norm:
                        f.setdefault(2, []).append(pending_norm.popleft())
                elif len(pending_norm) >= 2:
                    f.setdefault(3, []).insert(0, pending_norm.popleft())
                pending_norm.append(attn_head(h, f))
            while pending_norm:
                pending_norm.popleft()()

            for ct in range(CT):
                for qc in range(NQC):
                    proj7(ct, qc)

    nc.compile()
    return nc


def _get_nc():
    if "nc" not in _CACHE:
        _CACHE["nc"] = _build()
    return _CACHE["nc"]


def _prep_shards(x, w_qkv, w_proj, b_proj):
    bf16 = ml_dtypes.bfloat16
    x = np.asarray(x, dtype=np.float32)
    w_qkv = np.asarray(w_qkv, dtype=np.float32)
    w_proj = np.asarray(w_proj, dtype=np.float32)
    b_proj = np.asarray(b_proj, dtype=np.float32)

    def fmajor(w):  # [768c, 768f] -> [128, FT, CT, 128]: (p,f,ct,j) = w[ct*128+p, f*128+j]
        a = w.reshape(CT, 128, FT, 128)
        return np.ascontiguousarray(a.transpose(1, 2, 0, 3)).astype(bf16)

    def pmajor(w):  # [768c, F] -> [128, CT, F]
        return np.ascontiguousarray(
            w.reshape(CT, 128, w.shape[1]).transpose(1, 0, 2)
        ).astype(bf16)

    wq_b = fmajor(w_qkv[0:DIM].T * SCALE)
    wk_b = fmajor(w_qkv[DIM:2 * DIM].T)
    wv_b = pmajor(w_qkv[2 * DIM:3 * DIM].T)
    wp_arr = np.zeros((DH + 1, H, DIM), np.float32)
    wp_arr[0:DH] = w_proj.T.reshape(H, DH, DIM).transpose(1, 0, 2)
    wp_arr[DH, 0, :] = b_proj
    wp_b = np.ascontiguousarray(wp_arr).astype(bf16)

    in_maps = []
    for c in range(NCORES):
        b, half = divmod(c, 2)
        xt = x[b].T  # [768, 2048]
        if half == 1:
            xt = np.concatenate([xt[:, NQ:], xt[:, :NQ]], axis=1)
        xq = pmajor(xt)  # [128, CT, 2048]
        im = {"wk": wk_b, "wq": wq_b, "wv": wv_b, "wp": wp_b}
        for i in range(4):
            im[f"xq{i}"] = np.ascontiguousarray(xq[:, :, i * 512:(i + 1) * 512])
        in_maps.append(im)
    return in_maps


def kernel(x, w_qkv, w_proj, b_proj):
    from concourse.bass_utils import run_bass_kernel_spmd

    nc = _get_nc()
    in_maps = _prep_shards(x, w_qkv, w_proj, b_proj)
    res = run_bass_kernel_spmd(nc, in_maps, core_ids=list(range(NCORES)))
    out = np.empty((B, N, DIM), np.float32)
    for c in range(NCORES):
        b, half = divmod(c, 2)
        yT = np.asarray(res.results[c]["out"], dtype=np.float32)  # [768, 1024]
        out[b, half * NQ:(half + 1) * NQ, :] = yT.T
    return out
